# revision 1
# baseline (speedup 1.0000x reference)
"""Causal multi-head attention on 8 Trainium2 NeuronCores (Bass/Tile).

Sharding: tensor-parallel over heads (per the problem hint). Core i owns the
128 projected columns (2 heads x 64) [128*i, 128*(i+1)): Wq/Wk/Wv split
column-wise, Wo split row-wise. Each core computes a full-[T, D] partial of
the output projection; the host sums the 8 partials and adds bo (the
row-parallel unshard).

Per-core kernel (all matmuls in float32r -- fp32 data, fast PE mode):
  - QKV projections produce Q^T/K^T/V^T [128, S] per batch with biases fused
    into the PSUM->SBUF copy on the scalar engine (1/sqrt(dh) folded into Wq).
  - V^T is PE-transposed per 128-key chunk into V' [keys, 128] with a ones
    column appended, so the PV matmul also produces the softmax denominator.
  - Attention per 512-query tile over 128-key chunks: S^T = K^T.T @ Q^T for
    both heads concurrently (disjoint PE row groups via base partitions 0/64,
    one 2-bank PSUM tile), exp on ACT as a single double-width op (softmax
    without max-subtraction -- scores are bounded by construction), causal
    mask multiply only on diagonal chunks (fully-masked chunks are skipped,
    diagonal chunks compute only the valid query window), PV accumulates
    out^T and the denominator in PSUM.
  - Normalization: reciprocal of the denominator rows, partition-broadcast
    via a DRAM bounce, multiply into attnout^T.
  - Output projection attnout^T.T @ Wo_slice^T -> partial [T, D] -> DMA out.
Projections of batch b+1 are software-pipelined into the attention loop of
batch b to overlap PE-heavy and ACT-heavy phases.
"""
from contextlib import ExitStack

import numpy as np

import concourse.bass as bass
import concourse.mybir as mybir
import concourse.tile as tile
from concourse import bacc
from concourse.bass import ts, ds
from concourse.bass_utils import run_bass_kernel_spmd

F32 = mybir.dt.float32
F32R = mybir.dt.float32r
AF = mybir.ActivationFunctionType
MULT = mybir.AluOpType.mult

B, S, D = 4, 2048, 1024
P = 128
DH = 64
KO = D // P        # 8 contraction chunks for projections
QTILE = 512
CH = 128
TTILE = 512
N_CORES = 8


def _build_nc(reps=1):
    T = B * S
    n_ttiles_b = S // TTILE
    n_qt = S // QTILE
    n_ch = S // CH
    diag_per_q = QTILE // CH

    nc = bacc.Bacc()
    xT = nc.declare_dram_parameter("xT", [D, T], F32R, isOutput=False)
    wqT = nc.declare_dram_parameter("wqT", [D, P], F32R, isOutput=False)
    wkT = nc.declare_dram_parameter("wkT", [D, P], F32R, isOutput=False)
    wvT = nc.declare_dram_parameter("wvT", [D, P], F32R, isOutput=False)
    woT = nc.declare_dram_parameter("woT", [P, D], F32R, isOutput=False)
    bqv = nc.declare_dram_parameter("bq", [P, 1], F32, isOutput=False)
    bkv = nc.declare_dram_parameter("bk", [P, 1], F32, isOutput=False)
    bvv = nc.declare_dram_parameter("bv", [P, 1], F32, isOutput=False)
    cmask = nc.declare_dram_parameter("cmask", [P, diag_per_q, QTILE], F32R,
                                      isOutput=False)
    ident2 = nc.declare_dram_parameter("ident2", [P, DH], F32, isOutput=False)
    vpad1 = nc.declare_dram_parameter("vpad1", [P, P], F32R, isOutput=False)
    vpad2 = nc.declare_dram_parameter("vpad2", [P, P], F32R, isOutput=False)
    out = nc.declare_dram_parameter("out", [T, D], F32, isOutput=True)

    with tile.TileContext(nc) as tc, ExitStack() as ctx:
        const = ctx.enter_context(tc.tile_pool(name="const", bufs=1))
        bigp = ctx.enter_context(tc.tile_pool(name="big", bufs=2))
        xp = ctx.enter_context(tc.tile_pool(name="xp", bufs=2))
        ptp = ctx.enter_context(tc.tile_pool(name="pt", bufs=2))
        vp = ctx.enter_context(tc.tile_pool(name="vp", bufs=2))
        osp = ctx.enter_context(tc.tile_pool(name="os", bufs=2))
        drp = ctx.enter_context(tc.tile_pool(name="dr", bufs=2, space="DRAM"))
        ps = ctx.enter_context(tc.tile_pool(name="ps", bufs=2, space="PSUM"))

        wq_t = const.tile([P, KO, P], F32R, tag="wq")
        wk_t = const.tile([P, KO, P], F32R, tag="wk")
        wv_t = const.tile([P, KO, P], F32R, tag="wv")
        nc.sync.dma_start(out=wq_t, in_=wqT.rearrange("(ko ki) m -> ki ko m", ki=P))
        nc.sync.dma_start(out=wk_t, in_=wkT.rearrange("(ko ki) m -> ki ko m", ki=P))
        nc.sync.dma_start(out=wv_t, in_=wvT.rearrange("(ko ki) m -> ki ko m", ki=P))
        wo_t = const.tile([P, D], F32R, tag="wo")
        nc.sync.dma_start(out=wo_t, in_=woT[:, :])
        bq_t = const.tile([P, 1], F32, tag="bq")
        bk_t = const.tile([P, 1], F32, tag="bk")
        bv_t = const.tile([P, 1], F32, tag="bv")
        nc.sync.dma_start(out=bq_t, in_=bqv[:, :])
        nc.sync.dma_start(out=bk_t, in_=bkv[:, :])
        nc.sync.dma_start(out=bv_t, in_=bvv[:, :])
        cm_t = const.tile([P, diag_per_q, QTILE], F32R, tag="cm")
        nc.sync.dma_start(out=cm_t, in_=cmask[:, :, :])
        ident = const.tile([P, DH], F32, tag="id")
        nc.sync.dma_start(out=ident, in_=ident2[:, :])

        xT_r = xT.rearrange("(ko ki) t -> ki ko t", ki=P)

        rep_ctx = tc.For_i(0, reps, 1) if reps > 1 else None
        if rep_ctx is not None:
            ctx.enter_context(rep_ctx)

        pb = {}
        vb = {}
        ab_ = {}

        def alloc_proj(b):
            qt_b = bigp.tile([P, S], F32R, tag="qt", name=f"qt{b}")
            kt_b = bigp.tile([P, S], F32R, tag="kt", name=f"kt{b}")
            vt_b = bigp.tile([P, S], F32, tag="vt", name=f"vt{b}")
            pb[b] = (qt_b, kt_b, vt_b)

        def proj_ttile(b, tt):
            qt_b, kt_b, vt_b = pb[b]
            b0 = b * S
            xt = xp.tile([P, KO, TTILE], F32R, tag="xt", name=f"xt{b}_{tt}")
            nc.sync.dma_start(out=xt, in_=xT_r[:, :, ds(b0 + tt * TTILE, TTILE)])
            for pi, (w_t, b_t, dst) in enumerate((
                    (wq_t, bq_t, qt_b), (wk_t, bk_t, kt_b), (wv_t, bv_t, vt_b))):
                psm = ps.tile([P, TTILE], F32, tag="s12",
                              name=f"psm{b}_{tt}_{pi}")
                for ko in range(KO):
                    nc.tensor.matmul(psm, w_t[:, ko], xt[:, ko],
                                     start=(ko == 0), stop=(ko == KO - 1))
                nc.scalar.activation(out=dst[:, ts(tt, TTILE)], in_=psm,
                                     func=AF.Identity, bias=b_t, scale=1.0)

        def vbuild(b):
            vt_b = pb[b][2]
            v1 = vp.tile([P, n_ch, P], F32R, tag="v1", name=f"v1_{b}")
            v2 = vp.tile([P, n_ch, P], F32R, tag="v2", name=f"v2_{b}")
            vb[b] = (v1, v2)
            vp1_b = bass.AP(tensor=vpad1, offset=0, ap=[[P, P], [0, n_ch], [1, P]])
            vp2_b = bass.AP(tensor=vpad2, offset=0, ap=[[P, P], [0, n_ch], [1, P]])
            nc.sync.dma_start(out=v1, in_=vp1_b)
            nc.sync.dma_start(out=v2, in_=vp2_b)
            for c in range(n_ch):
                tp1 = ps.tile([P, DH], F32, tag="s12", name=f"tp1_{b}_{c}")
                nc.tensor.transpose(tp1, vt_b[0:DH, ts(c, CH)], ident[0:DH])
                nc.vector.tensor_copy(out=v1[:, c, 0:DH], in_=tp1)
                tp2 = ps.tile([P, DH], F32, tag="s12", name=f"tp2_{b}_{c}")
                nc.tensor.transpose(tp2, vt_b[DH:P, ts(c, CH)], ident[DH:P])
                nc.vector.tensor_copy(out=v2[:, c, DH:P], in_=tp2)

        def attn_qtile(b, j):
            qt_b, kt_b, _ = pb[b]
            v1, v2 = vb[b]
            ao_b = ab_[b]
            qsl = ds(j * QTILE, QTILE)
            o1 = ps.tile([P, QTILE], F32, tag="o1", name=f"o1_{b}_{j}")
            o2 = ps.tile([P, QTILE], F32, tag="o2", name=f"o2_{b}_{j}")
            nch_j = (j + 1) * QTILE // CH
            for c in range(nch_j):
                di = c - j * diag_per_q
                off = max(0, di) * CH
                qs = ds(j * QTILE + off, QTILE - off)
                s12 = ps.tile([P, 2, QTILE], F32, tag="s12", name=f"s12_{b}_{j}_{c}")
                ksl = ds(c * CH, CH)
                nc.tensor.matmul(s12[:, 0, off:], kt_b[0:DH, ksl], qt_b[0:DH, qs],
                                 start=True, stop=True)
                nc.tensor.matmul(s12[:, 1, off:], kt_b[DH:P, ksl], qt_b[DH:P, qs],
                                 start=True, stop=True)
                p12 = ptp.tile([P, 2, QTILE], F32R, tag="p12", bufs=3,
                               name=f"p12_{b}_{j}_{c}")
                nc.scalar.activation(out=p12[:, :, off:], in_=s12[:, :, off:],
                                     func=AF.Exp)
                if di >= 0:
                    cmb = bass.AP(tensor=cm_t.tensor,
                                  offset=cm_t[:, di, off:].offset,
                                  ap=[cm_t.ap[0], [0, 2], [1, QTILE - off]])
                    nc.vector.tensor_tensor(out=p12[:, :, off:],
                                            in0=p12[:, :, off:],
                                            in1=cmb, op=MULT)
                st, sp = (c == 0), (c == nch_j - 1)
                nc.tensor.matmul(o1[:, off:], v1[:, c], p12[:, 0, off:],
                                 start=st, stop=sp)
                nc.tensor.matmul(o2[:, off:], v2[:, c], p12[:, 1, off:],
                                 start=st, stop=sp)
            st1 = ptp.tile([P, QTILE], F32, tag="st1", name=f"st1_{b}_{j}")
            st2 = ptp.tile([P, QTILE], F32, tag="st2", name=f"st2_{b}_{j}")
            nc.vector.reciprocal(out=st1[DH:DH + 1], in_=o1[DH:DH + 1])
            nc.vector.reciprocal(out=st2[32:33], in_=o2[32:33])
            dsb = ptp.tile([P, QTILE], F32, tag="dsb", name=f"dsb_{b}_{j}")
            scr1 = drp.tile([1, QTILE], F32, tag="sc1", name=f"sc1_{b}_{j}")
            scr2 = drp.tile([1, QTILE], F32, tag="sc2", name=f"sc2_{b}_{j}")
            nc.sync.dma_start(out=scr1, in_=st1[DH:DH + 1])
            nc.sync.dma_start(out=scr2, in_=st2[32:33])
            nc.sync.dma_start(
                out=dsb[0:DH],
                in_=bass.AP(tensor=scr1.tensor, offset=scr1.offset,
                            ap=[[0, DH], [1, QTILE]]))
            nc.sync.dma_start(
                out=dsb[DH:P],
                in_=bass.AP(tensor=scr2.tensor, offset=scr2.offset,
                            ap=[[0, DH], [1, QTILE]]))
            nc.vector.tensor_tensor(out=ao_b[0:DH, qsl], in0=o1[0:DH],
                                    in1=dsb[0:DH], op=MULT)
            nc.vector.tensor_tensor(out=ao_b[DH:P, qsl], in0=o2[DH:P],
                                    in1=dsb[DH:P], op=MULT)

        def outproj(b):
            ao_b = ab_[b]
            b0 = b * S
            for tt in range(S // P):
                for nn in range(D // QTILE):
                    po = ps.tile([P, QTILE], F32, tag="s12",
                                 name=f"po{b}_{tt}_{nn}")
                    nc.tensor.matmul(po, ao_b[:, ts(tt, P)],
                                     wo_t[:, ts(nn, QTILE)], start=True, stop=True)
                    ot = osp.tile([P, QTILE], F32, tag=f"ot{nn % 2}", bufs=6,
                                  name=f"ot{b}_{tt}_{nn}")
                    if nn % 2 == 0:
                        nc.vector.tensor_copy(out=ot, in_=po)
                    else:
                        nc.scalar.copy(out=ot, in_=po)
                    nc.sync.dma_start(
                        out=out[ds(b0 + tt * P, P), ts(nn, QTILE)], in_=ot)

        alloc_proj(0)
        for tt in range(n_ttiles_b):
            proj_ttile(0, tt)
        for b in range(B):
            vbuild(b)
            if b + 1 < B:
                alloc_proj(b + 1)
            ao_b = bigp.tile([P, S], F32R, tag="ao", name=f"ao{b}")
            ab_[b] = ao_b
            for j in range(n_qt):
                if b + 1 < B and j < n_ttiles_b:
                    proj_ttile(b + 1, j)
                attn_qtile(b, j)
            outproj(b)
            pb.pop(b); vb.pop(b); ab_.pop(b)

    nc.compile()
    return nc


def _host_prepare(x, Wq, bq, Wk, bk, Wv, bv, Wo, bo):
    T = B * S
    scale = np.float32(1.0 / np.sqrt(np.float32(DH)))
    xT = np.ascontiguousarray(np.asarray(x, np.float32).reshape(T, D).T)

    k_idx = np.arange(CH)[:, None]
    q_idx = np.arange(QTILE)[None, :]
    cmaskv = np.stack(
        [(k_idx <= q_idx - off) for off in range(0, QTILE, CH)], axis=1
    ).astype(np.float32)

    vp1 = np.zeros((P, P), np.float32); vp1[:, DH] = 1.0
    vp2 = np.zeros((P, P), np.float32); vp2[:, 32] = 1.0
    identv = np.vstack([np.eye(DH, dtype=np.float32)] * 2)

    Wq = np.asarray(Wq, np.float32); Wk = np.asarray(Wk, np.float32)
    Wv = np.asarray(Wv, np.float32); Wo = np.asarray(Wo, np.float32)
    bq = np.asarray(bq, np.float32); bk = np.asarray(bk, np.float32)
    bv = np.asarray(bv, np.float32)

    in_maps = []
    for i in range(N_CORES):
        sl = slice(i * P, (i + 1) * P)
        in_maps.append({
            "xT": xT,
            "wqT": np.ascontiguousarray(Wq[sl].T * scale),
            "wkT": np.ascontiguousarray(Wk[sl].T),
            "wvT": np.ascontiguousarray(Wv[sl].T),
            "woT": np.ascontiguousarray(Wo[:, sl].T),
            "bq": (bq[sl] * scale).reshape(P, 1),
            "bk": bk[sl].reshape(P, 1).copy(),
            "bv": bv[sl].reshape(P, 1).copy(),
            "cmask": cmaskv,
            "ident2": identv,
            "vpad1": vp1,
            "vpad2": vp2,
        })
    return in_maps


_NC_CACHE = {}


def kernel(x, Wq, bq, Wk, bk, Wv, bv, Wo, bo):
    if "nc" not in _NC_CACHE:
        _NC_CACHE["nc"] = _build_nc()
    nc = _NC_CACHE["nc"]
    in_maps = _host_prepare(x, Wq, bq, Wk, bk, Wv, bv, Wo, bo)
    res = run_bass_kernel_spmd(nc, in_maps, core_ids=list(range(N_CORES)))
    acc = res.results[0]["out"].astype(np.float32).copy()
    for r in res.results[1:]:
        acc += r["out"]
    acc += np.asarray(bo, np.float32)
    return acc.reshape(B, S, D)
